# revision 28
# baseline (speedup 1.0000x reference)
"""Mesh vertex-normals kernel for 8 trn2 NeuronCores (Bass/Tile), v3.

Same math as the verified v2 baseline (structured-triangulation stencil),
restructured for speed:

  * SoA f32 layout: x/y/z planes are separate [rows, cols] tiles, so
    every element-wise op is unit-stride full-width (no stride-3 AoS
    penalties of v2).
  * 4x2 core grid instead of 8x1 bands: per-core 365 rows pack into
    3 x 128-partition blocks (vs 184 rows -> 2 blocks at 73% partition
    use). Engine time scales with free-dim size only, so fewer row
    blocks x wider tiles = ~25% less element-wise work.
  * The scatter stencil largely runs on the (otherwise idle) tensor
    engine via exact fp32 matmuls with identity/shifted-identity
    weights, accumulated in PSUM:
        s[r][j] = t[r+1][j+1] + c1[r+1][j] + t[r][j] + c2[r][j+1]
    is 4 accumulating matmuls per component; p/q/s of v2 disappear
    from the vector/pool engines. (fp16 data and float32r matmuls were
    both tried and REJECTED: min |s| over the mesh is 0.023, so any
    16-bit rounding of the geometry or the c-terms blows past the 2e-2
    gate at small-|s| vertices. fp32 matmul is exact - v2's 2.9e-5
    total error proves it.)
  * Output is stored fp16 (normals are unit vectors; 5e-4 quantization)
    and converted to f32 on host, halving output DMA.

Per-quad math (identical to the reference for the structured mesh):
    ex = v[:, c+1] - v[:, c]          horizontal edges
    ey = v[r+1] - v[r]                vertical edges (PE: (SH - I) @ v)
    dd = ex + ey[:, c+1]              quad diagonal (in-place over ex)
    c1 = cross(ex, ey[:, c+1])        tri1 weighted normal x2
    c2 = cross(dd, ey[:, c])          tri2 weighted normal x2
    t  = c1 + c2
    s  = 4-term shifted sum of t/c1/c2 (PE matmuls, PSUM f32)
    normal = s / sqrt(sum_k s_k^2 + eps)

Edge-duplicated padding makes all phantom quads outside the grid
contribute exactly zero (c1: zero horizontal edge; c2: cross of two
bit-identical vectors), so boundaries need no special-casing.

If `faces` does not match the structured triangulation (it always does
for the reference setup_inputs), falls back to an exact host computation.
"""

import sys

sys.path.insert(0, "/opt/trn_rl_repo")

import numpy as np

GRID = 1449
N_CORES = 8
PR, PC = 4, 2            # core grid: 4 row-bands x 2 col-halves
BASE_R = (GRID - 1) // PR   # 362 quad rows per core
BASE_C = (GRID - 1) // PC   # 724 quad cols per core
OUT_R, OUT_C = BASE_R + 1, BASE_C + 1   # 363, 725 output vertices
# fp32r matmuls need even moving sizes: compute one extra device column
DEV_C = OUT_C + 1                       # 726 compute cols (even)
IN_R, IN_C = BASE_R + 3, BASE_C + 4     # 365, 728 padded input vertices


# ---------------------------------------------------------------------------
# host-side helpers
# ---------------------------------------------------------------------------

def _is_structured(faces: np.ndarray, grid: int) -> bool:
    n_quads = (grid - 1) * (grid - 1)
    if faces.shape != (2 * n_quads, 3):
        return False
    idx = np.arange(grid * grid, dtype=np.int64).reshape(grid, grid)
    i00 = idx[:-1, :-1].ravel()
    i01 = idx[:-1, 1:].ravel()
    i10 = idx[1:, :-1].ravel()
    i11 = idx[1:, 1:].ravel()
    f = faces
    return (
        np.array_equal(f[:n_quads, 0], i00)
        and np.array_equal(f[:n_quads, 1], i01)
        and np.array_equal(f[:n_quads, 2], i11)
        and np.array_equal(f[n_quads:, 0], i00)
        and np.array_equal(f[n_quads:, 1], i11)
        and np.array_equal(f[n_quads:, 2], i10)
    )


def _host_fallback(vertices: np.ndarray, faces: np.ndarray) -> np.ndarray:
    """Exact replica of the reference for non-structured faces."""
    n_vertices = vertices.shape[0]
    va = vertices[faces[:, 0]]
    vb = vertices[faces[:, 1]]
    vc = vertices[faces[:, 2]]
    cross = np.cross(vb - va, vc - vb).astype(np.float32)
    norm = np.linalg.norm(cross, axis=-1, keepdims=True)
    weighted = (cross / norm) * (norm * 0.5)
    data = np.broadcast_to(weighted[:, None, :], (faces.shape[0], 3, 3)).reshape(-1, 3)
    summed = np.zeros((n_vertices, 3), dtype=np.float32)
    np.add.at(summed, faces.reshape(-1), data)
    norms = np.linalg.norm(summed, axis=-1, keepdims=True)
    return (summed / np.maximum(norms, 1e-10)).astype(np.float32)


def _row_blocks(n_v_rows: int):
    """Split a band's vertex rows into <=128-partition blocks, overlap 2."""
    blocks = []
    r0 = 0
    while r0 < n_v_rows - 2:
        nv = min(128, n_v_rows - r0)
        blocks.append((r0, nv))
        r0 += nv - 2
    return blocks


def _col_chunks(width: int, chunk: int):
    return [(c0, min(chunk, width - c0)) for c0 in range(0, width, chunk)]


# ---------------------------------------------------------------------------
# device program
# ---------------------------------------------------------------------------

DEFAULT_CFG = dict(
    col_chunk=364,    # output columns per unit
    psum_cols=364,    # columns per PSUM bank chunk (f32, <=512)
    dd_inplace=True,  # dd overwrites ex (measured best; separate tiles -1.5%)
    io_bufs=3,
    wk_bufs=3,
    psv_bufs=2,
    ps_bufs=6,
    dma_splits=1,
    dma_splits_o=1,
    store_eng='scalar',  # output stores from Act's DMA queue (SP issues loads)
    # engine per op: 'v' = vector (DVE), 'g' = gpsimd (Pool), 'a' = Act
    # sq: 'a' -> Act Square(psum); 'v'/'g' -> tensor mult from psum
    eng=dict(ex='v', dd='g', mm='v', c1s='v', c2s='g', t='v',
             ns1='v', ns2='v', o='v', sq='a'),
)


def _cfg_key(cfg):
    return (cfg["col_chunk"], cfg["psum_cols"], cfg["io_bufs"],
            cfg["wk_bufs"], cfg["psv_bufs"], cfg["ps_bufs"],
            cfg.get("dma_splits", 1), cfg.get("dma_splits_o", 1),
            cfg.get("vy_dma", False), cfg.get("accum3", False),
            cfg.get("dd_inplace", False), cfg.get("split_frac", 0.72),
            cfg.get("store_eng", "sync"), cfg.get("fused_io", False),
            tuple(sorted(cfg["eng"].items())))


def _build_program(grid: int, n_cores: int, repeats: int = 1, cfg=None):
    import contextlib

    import concourse.bacc as bacc
    import concourse.tile as tile
    from concourse import mybir
    from concourse.masks import make_identity

    cfg = cfg or DEFAULT_CFG
    f32 = mybir.dt.float32
    f16 = mybir.dt.float16

    nc = bacc.Bacc()
    vband = nc.dram_tensor("vband", [3, IN_R, IN_C], f32,
                           kind="ExternalInput")
    oband = nc.dram_tensor("oband", [3, OUT_R, OUT_C], f16,
                           kind="ExternalOutput")
    dbg = {}
    for tag in cfg.get("dbg", ()):
        dbg[tag] = nc.dram_tensor(f"dbg_{tag}", [128, 1024], f32,
                                  kind="ExternalOutput")

    with tile.TileContext(nc) as tc:
        with (
            tc.tile_pool(name="io", bufs=cfg["io_bufs"]) as io,
            tc.tile_pool(name="wk", bufs=cfg["wk_bufs"]) as wk,
            tc.tile_pool(name="psv", bufs=cfg["psv_bufs"], space="PSUM") as psv,
            tc.tile_pool(name="ps", bufs=cfg["ps_bufs"], space="PSUM") as psp,
            tc.tile_pool(name="cst", bufs=1) as cst,
        ):
            eps_tile = cst.tile([128, 1], f32, tag="eps")
            nc.vector.memset(eps_tile[:, :], 1e-12)
            # f32 staging masks (memset/affine_select reject float32r),
            # then gpsimd cast-DMA into the float32r weight tiles.
            # tid[:, :128] = I; cols 128/129 zero => tid[:, 1:129] is the
            # up-shift matrix SH[k, m] = 1 iff k == m+1.
            tid32 = cst.tile([128, 130], f32, tag="tid32")
            nc.gpsimd.memset(tid32[:, :], 0.0)
            make_identity(nc, tid32[:, 0:128], nomemset=True)
            # tmix[:, 1:129][k, m] = +1 if k == m+1 else (-1 if k == m)
            tmix32 = cst.tile([128, 130], f32, tag="tmix32")
            nc.gpsimd.memset(tmix32[:, :], 0.0)
            make_identity(nc, tmix32[:, 0:128], nomemset=True)
            nc.gpsimd.affine_select(
                out=tmix32[:, 1:129], in_=tmix32[:, 1:129],
                compare_op=mybir.AluOpType.not_equal, fill=-1.0, base=0,
                pattern=[[-1, 128]], channel_multiplier=1,
            )
            tid, tmix = tid32, tmix32

            loop = tc.For_i(0, repeats, 1) if repeats > 1 else contextlib.nullcontext()
            with loop:
                _emit_body(nc, io, wk, psv, psp, eps_tile, tid, tmix,
                           vband, oband, mybir, cfg, dbg)

    nc.finalize()
    return nc


def _emit_body(nc, io, wk, psv, psp, eps_tile, tid, tmix, vband, oband,
               mybir, cfg, dbg=None):
    dbg = dbg or {}
    f32r = mybir.dt.float32r

    def dump(tag, tile, rows, cols):
        if tag in dbg:
            nc.sync.dma_start(out=dbg[tag][0:rows, 0:cols],
                              in_=tile[0:rows, 0:cols])
    f32 = mybir.dt.float32
    f16 = mybir.dt.float16
    Alu = mybir.AluOpType
    Act = mybir.ActivationFunctionType
    ENG = {"v": nc.vector, "g": nc.gpsimd, "s": None}
    eng = {k: ENG.get(v) for k, v in cfg["eng"].items()}

    frac = cfg.get("split_frac", 0.72)

    def tt(e, out, in0, in1, op):
        if e is not None:
            e.tensor_tensor(out=out, in0=in0, in1=in1, op=op)
            return
        # split the op by columns across DVE and Pool
        kk = int(out.shape[1] * frac)
        nc.vector.tensor_tensor(out=out[:, 0:kk], in0=in0[:, 0:kk],
                                in1=in1[:, 0:kk], op=op)
        nc.gpsimd.tensor_tensor(out=out[:, kk:], in0=in0[:, kk:],
                                in1=in1[:, kk:], op=op)

    nsp = cfg.get("dma_splits", 1)
    osp = cfg.get("dma_splits_o", 1)

    def split_dma(out_tile, in_ap, rows, splits):
        if splits <= 1:
            nc.sync.dma_start(out=out_tile, in_=in_ap)
            return
        step = (rows + splits - 1) // splits
        for p0 in range(0, rows, step):
            p1 = min(p0 + step, rows)
            nc.sync.dma_start(out=out_tile[p0:p1], in_=in_ap[p0:p1])

    for r0, nv in _row_blocks(IN_R):
        nq = nv - 1   # quad rows in this block
        ns = nv - 2   # output rows in this block
        for c0, w in _col_chunks(DEV_C, cfg["col_chunk"]):
            wi = w + 2   # input columns
            # ---- load the three coordinate planes -------------------------
            if cfg.get("fused_io", False):
                v3 = io.tile([nv, 3, wi], f32, tag="v3", name="v3")
                nc.sync.dma_start(
                    out=v3[:, :, :],
                    in_=vband[0:3, r0:r0 + nv,
                              c0:c0 + wi].transpose([1, 0, 2]))
                v = [v3[:, c, :] for c in range(3)]
            else:
                v = [io.tile([nv, wi], f32, tag=f"v{c}", name=f"v{c}")
                     for c in range(3)]
                for c in range(3):
                    split_dma(v[c][:, :], vband[c, r0:r0 + nv, c0:c0 + wi],
                              nv, nsp)
            # ---- vertical edges -------------------------------------------
            ey = [wk.tile([nq, wi], f32, tag=f"ey{c}", name=f"ey{c}")
                  for c in range(3)]
            if cfg.get("vy_dma", False):
                vd = [io.tile([nq, wi], f32, tag=f"vd{c}", name=f"vd{c}")
                      for c in range(3)]
                for c in range(3):
                    split_dma(vd[c][:, :],
                              vband[c, r0 + 1:r0 + nv, c0:c0 + wi], nq, nsp)
                    tt(eng.get("ey", nc.vector), ey[c][:, :], vd[c][:, :],
                       v[c][0:nq, :], Alu.subtract)
            else:
                for c in range(3):
                    for j0, pw in _col_chunks(wi, cfg["psum_cols"]):
                        pv = psv.tile([128, pw], f32, tag="psv")
                        nc.tensor.matmul(
                            out=pv[:, :], lhsT=tmix[0:nv, 1:129],
                            rhs=v[c][:, j0:j0 + pw], start=True, stop=True,
                        )
                        nc.scalar.activation(
                            out=ey[c][:, j0:j0 + pw], in_=pv[0:nq, :],
                            func=Act.Copy,
                        )
            # ---- horizontal edges -----------------------------------------
            # ---- horizontal edges -----------------------------------------
            # ---- horizontal edges -----------------------------------------
            ex = [wk.tile([nq, w + 1], f32, tag=f"ex{c}", name=f"ex{c}") for c in range(3)]
            for c in range(3):
                tt(eng["ex"], ex[c][:, :], v[c][0:nq, 1:wi],
                   v[c][0:nq, 0:w + 1], Alu.subtract)

            # ---- cross products -------------------------------------------
            # c1 = cross(ex, eyR); then ex becomes dd = ex + eyR in place;
            # c2 = cross(dd, eyL). One scratch plane per component.
            c1 = [wk.tile([nq, w + 1], f32, tag=f"c1{c}", name=f"c1{c}") for c in range(3)]
            c2 = [wk.tile([nq, w + 1], f32, tag=f"c2{c}", name=f"c2{c}") for c in range(3)]
            m2 = [wk.tile([nq, w + 1], f32, tag=f"m2{c}", name=f"m2{c}") for c in range(3)]
            for k in range(3):
                u, x = (k + 1) % 3, (k + 2) % 3
                tt(eng["mm"], c1[k][:, :], ex[u][:, :], ey[x][:, 1:wi],
                   Alu.mult)
                tt(eng["mm"], m2[k][:, :], ex[x][:, :], ey[u][:, 1:wi],
                   Alu.mult)
                tt(eng["c1s"], c1[k][:, :], c1[k][:, :], m2[k][:, :],
                   Alu.subtract)
            if cfg.get("dd_inplace", False):
                dd = ex
            else:
                dd = [wk.tile([nq, w + 1], f32, tag=f"dd{c}", name=f"dd{c}")
                      for c in range(3)]
            for c in range(3):
                tt(eng["dd"], dd[c][:, :], ex[c][:, :], ey[c][:, 1:wi],
                   Alu.add)
            for k in range(3):
                u, x = (k + 1) % 3, (k + 2) % 3
                tt(eng["mm"], c2[k][:, :], dd[u][:, :], ey[x][:, 0:w + 1],
                   Alu.mult)
                tt(eng["mm"], m2[k][:, :], dd[x][:, :], ey[u][:, 0:w + 1],
                   Alu.mult)
                tt(eng["c2s"], c2[k][:, :], c2[k][:, :], m2[k][:, :],
                   Alu.subtract)

            dump("c10", c1[0], nq, w + 1)
            dump("c20", c2[0], nq, w + 1)
            # ---- 4-term exact fp32 PE accumulation ------------------------
            # s[r][j] = t[r+1][j+1] + c1[r+1][j] + t[r][j] + c2[r][j+1]
            t = [wk.tile([nq, w + 1], f32, tag=f"t{c}", name=f"t{c}")
                 for c in range(3)]
            for k in range(3):
                tt(eng["t"], t[k][:, :], c1[k][:, :], c2[k][:, :], Alu.add)
            if cfg.get("fused_io", False):
                o3 = io.tile([ns, 3, w], f16, tag="o3", name="o3")
            accum3 = cfg.get("accum3", False)
            if accum3:
                p = [wk.tile([nq, w], f32, tag=f"p{c}", name=f"p{c}")
                     for c in range(3)]
                for k in range(3):
                    tt(eng.get("p", nc.gpsimd), p[k][:, :], t[k][:, 1:w + 1],
                       c1[k][:, 0:w], Alu.add)
            if cfg.get("fused_io", False):
                o = [o3[:, c, :] for c in range(3)]
            else:
                o = [io.tile([ns, w], f16, tag=f"o{c}", name=f"o{c}")
                     for c in range(3)]
            I_, SH = tid[0:nq, 0:128], tid[0:nq, 1:129]
            for j0, pw in _col_chunks(w, cfg["psum_cols"]):
                ps = [psp.tile([128, pw], f32, tag="ps", name="ps") for _ in range(3)]
                # grouped by stationary weight to avoid per-matmul reloads
                if accum3:
                    mm_seq = [(SH, k, p[k][:, j0:j0 + pw]) for k in range(3)]
                    nterms = 3
                else:
                    mm_seq = [(SH, k, t[k][:, j0 + 1:j0 + 1 + pw])
                              for k in range(3)]
                    mm_seq += [(SH, k, c1[k][:, j0:j0 + pw]) for k in range(3)]
                    nterms = 4
                mm_seq += [(I_, k, t[k][:, j0:j0 + pw]) for k in range(3)]
                mm_seq += [(I_, k, c2[k][:, j0 + 1:j0 + 1 + pw])
                           for k in range(3)]
                seen = [0, 0, 0]
                for lhs, k, rhs in mm_seq:
                    seen[k] += 1
                    nc.tensor.matmul(out=ps[k][:, :], lhsT=lhs, rhs=rhs,
                                     start=(seen[k] == 1),
                                     stop=(seen[k] == nterms))
                sq = [wk.tile([ns, pw], f32, tag=f"sq{c}", name=f"sq{c}") for c in range(3)]
                for k in range(3):
                    if cfg["eng"]["sq"] == "a":
                        nc.scalar.activation(out=sq[k][:, :],
                                             in_=ps[k][0:ns, :],
                                             func=Act.Square)
                    else:
                        tt(eng["sq"], sq[k][:, :], ps[k][0:ns, :],
                           ps[k][0:ns, :], Alu.mult)
                dump("sq0", sq[0], ns, pw)
                nsq = wk.tile([ns, pw], f32, tag="nsq")
                tt(eng["ns1"], nsq[:, :], sq[0][:, :], sq[1][:, :], Alu.add)
                tt(eng["ns2"], nsq[:, :], nsq[:, :], sq[2][:, :], Alu.add)
                dump("nsq", nsq, ns, pw)
                rn = wk.tile([ns, pw], f32, tag="rn")
                nc.scalar.activation(out=rn[:, :], in_=nsq[:, :],
                                     func=Act.Sqrt, bias=eps_tile[:ns, :])
                if cfg["eng"].get("rcp", "v") == "v":
                    nc.vector.reciprocal(out=rn[:, :], in_=rn[:, :])
                else:
                    nc.gpsimd.reciprocal(out=rn[:, :], in_=rn[:, :])
                dump("rn", rn, ns, pw)
                for k in range(3):
                    tt(eng["o"], o[k][:, j0:j0 + pw], ps[k][0:ns, :],
                       rn[:, :], Alu.mult)
            dump("o0", o[0], ns, w)
            ws = min(w, OUT_C - c0)
            st = {"sync": nc.sync, "scalar": nc.scalar,
                  "vector": nc.vector}[cfg.get("store_eng", "sync")]
            if cfg.get("fused_io", False):
                st.dma_start(
                    out=oband[0:3, r0:r0 + ns,
                              c0:c0 + ws].transpose([1, 0, 2]),
                    in_=o3[:, :, 0:ws])
                continue
            for k in range(3):
                if osp <= 1:
                    st.dma_start(out=oband[k, r0:r0 + ns, c0:c0 + ws],
                                 in_=o[k][:, 0:ws])
                else:
                    step = (ns + osp - 1) // osp
                    for p0 in range(0, ns, step):
                        p1 = min(p0 + step, ns)
                        st.dma_start(
                            out=oband[k, r0 + p0:r0 + p1, c0:c0 + ws],
                            in_=o[k][p0:p1, 0:ws])


_PROGRAM_CACHE: dict = {}


def _get_program(grid: int, n_cores: int, repeats: int = 1, cfg=None):
    cfg = cfg or DEFAULT_CFG
    key = (grid, n_cores, repeats, _cfg_key(cfg))
    if key not in _PROGRAM_CACHE:
        _PROGRAM_CACHE[key] = _build_program(grid, n_cores, repeats, cfg)
    return _PROGRAM_CACHE[key]


def _shard_inputs(vertices: np.ndarray):
    """fp16 SoA planes, edge-padded, sliced into the 4x2 core grid."""
    G = GRID
    V3 = vertices.reshape(G, G, 3)
    VP = np.pad(V3, ((1, 1), (1, 2), (0, 0)), mode="edge")
    VPs = np.ascontiguousarray(VP.transpose(2, 0, 1))   # [3, G+2, G+2]
    in_maps = []
    for R in range(PR):
        for C in range(PC):
            sl = VPs[:, R * BASE_R:R * BASE_R + IN_R,
                     C * BASE_C:C * BASE_C + IN_C]
            in_maps.append({"vband": np.ascontiguousarray(sl)})
    return in_maps


def _gather_output(results):
    G = GRID
    out = np.empty((G, G, 3), dtype=np.float32)
    k = 0
    for R in range(PR):
        for C in range(PC):
            ob = results[k]["oband"]    # [3, OUT_R, OUT_C] fp16
            tr = BASE_R if R < PR - 1 else OUT_R
            tc_ = BASE_C if C < PC - 1 else OUT_C
            out[R * BASE_R:R * BASE_R + tr,
                C * BASE_C:C * BASE_C + tc_, :] = (
                ob[:, :tr, :tc_].transpose(1, 2, 0).astype(np.float32))
            k += 1
    return out.reshape(G * G, 3)


def _run_stencil_on_device(vertices: np.ndarray, grid: int, n_cores: int,
                           trace: bool = False, repeats: int = 1, cfg=None):
    from concourse.bass_utils import run_bass_kernel_spmd

    in_maps = _shard_inputs(vertices)
    nc = _get_program(grid, n_cores, repeats, cfg)
    kres = run_bass_kernel_spmd(nc, in_maps, list(range(n_cores)),
                                trace=trace)
    return _gather_output(kres.results), kres


def kernel(vertices: np.ndarray, faces: np.ndarray) -> np.ndarray:
    vertices = np.asarray(vertices, dtype=np.float32)
    faces = np.asarray(faces)
    if (
        vertices.shape == (GRID * GRID, 3)
        and _is_structured(faces, GRID)
    ):
        out, _ = _run_stencil_on_device(vertices, GRID, N_CORES)
        return out
    print("kernel: faces are not the structured triangulation; host fallback",
          file=sys.stderr)
    return _host_fallback(vertices, faces)


# revision 29
# speedup vs baseline: 1.0024x; 1.0024x over previous
"""Mesh vertex-normals kernel for 8 trn2 NeuronCores (Bass/Tile), v3.

Same math as the verified v2 baseline (structured-triangulation stencil),
restructured for speed:

  * SoA f32 layout: x/y/z planes are separate [rows, cols] tiles, so
    every element-wise op is unit-stride full-width (no stride-3 AoS
    penalties of v2).
  * 4x2 core grid instead of 8x1 bands: per-core 365 rows pack into
    3 x 128-partition blocks (vs 184 rows -> 2 blocks at 73% partition
    use). Engine time scales with free-dim size only, so fewer row
    blocks x wider tiles = ~25% less element-wise work.
  * The scatter stencil largely runs on the (otherwise idle) tensor
    engine via exact fp32 matmuls with identity/shifted-identity
    weights, accumulated in PSUM:
        s[r][j] = t[r+1][j+1] + c1[r+1][j] + t[r][j] + c2[r][j+1]
    is 4 accumulating matmuls per component; p/q/s of v2 disappear
    from the vector/pool engines. (fp16 data and float32r matmuls were
    both tried and REJECTED: min |s| over the mesh is 0.023, so any
    16-bit rounding of the geometry or the c-terms blows past the 2e-2
    gate at small-|s| vertices. fp32 matmul is exact - v2's 2.9e-5
    total error proves it.)
  * Output is stored fp16 (normals are unit vectors; 5e-4 quantization)
    and converted to f32 on host, halving output DMA.

Per-quad math (identical to the reference for the structured mesh):
    ex = v[:, c+1] - v[:, c]          horizontal edges
    ey = v[r+1] - v[r]                vertical edges (PE: (SH - I) @ v)
    dd = ex + ey[:, c+1]              quad diagonal (in-place over ex)
    c1 = cross(ex, ey[:, c+1])        tri1 weighted normal x2
    c2 = cross(dd, ey[:, c])          tri2 weighted normal x2
    t  = c1 + c2
    s  = 4-term shifted sum of t/c1/c2 (PE matmuls, PSUM f32)
    normal = s / sqrt(sum_k s_k^2 + eps)

Edge-duplicated padding makes all phantom quads outside the grid
contribute exactly zero (c1: zero horizontal edge; c2: cross of two
bit-identical vectors), so boundaries need no special-casing.

If `faces` does not match the structured triangulation (it always does
for the reference setup_inputs), falls back to an exact host computation.
"""

import sys

sys.path.insert(0, "/opt/trn_rl_repo")

import numpy as np

GRID = 1449
N_CORES = 8
PR, PC = 4, 2            # core grid: 4 row-bands x 2 col-halves
BASE_R = (GRID - 1) // PR   # 362 quad rows per core
BASE_C = (GRID - 1) // PC   # 724 quad cols per core
OUT_R, OUT_C = BASE_R + 1, BASE_C + 1   # 363, 725 output vertices
# fp32r matmuls need even moving sizes: compute one extra device column
DEV_C = OUT_C + 1                       # 726 compute cols (even)
IN_R, IN_C = BASE_R + 3, BASE_C + 4     # 365, 728 padded input vertices


# ---------------------------------------------------------------------------
# host-side helpers
# ---------------------------------------------------------------------------

def _is_structured(faces: np.ndarray, grid: int) -> bool:
    n_quads = (grid - 1) * (grid - 1)
    if faces.shape != (2 * n_quads, 3):
        return False
    idx = np.arange(grid * grid, dtype=np.int64).reshape(grid, grid)
    i00 = idx[:-1, :-1].ravel()
    i01 = idx[:-1, 1:].ravel()
    i10 = idx[1:, :-1].ravel()
    i11 = idx[1:, 1:].ravel()
    f = faces
    return (
        np.array_equal(f[:n_quads, 0], i00)
        and np.array_equal(f[:n_quads, 1], i01)
        and np.array_equal(f[:n_quads, 2], i11)
        and np.array_equal(f[n_quads:, 0], i00)
        and np.array_equal(f[n_quads:, 1], i11)
        and np.array_equal(f[n_quads:, 2], i10)
    )


def _host_fallback(vertices: np.ndarray, faces: np.ndarray) -> np.ndarray:
    """Exact replica of the reference for non-structured faces."""
    n_vertices = vertices.shape[0]
    va = vertices[faces[:, 0]]
    vb = vertices[faces[:, 1]]
    vc = vertices[faces[:, 2]]
    cross = np.cross(vb - va, vc - vb).astype(np.float32)
    norm = np.linalg.norm(cross, axis=-1, keepdims=True)
    weighted = (cross / norm) * (norm * 0.5)
    data = np.broadcast_to(weighted[:, None, :], (faces.shape[0], 3, 3)).reshape(-1, 3)
    summed = np.zeros((n_vertices, 3), dtype=np.float32)
    np.add.at(summed, faces.reshape(-1), data)
    norms = np.linalg.norm(summed, axis=-1, keepdims=True)
    return (summed / np.maximum(norms, 1e-10)).astype(np.float32)


def _row_blocks(n_v_rows: int):
    """Split a band's vertex rows into <=128-partition blocks, overlap 2."""
    blocks = []
    r0 = 0
    while r0 < n_v_rows - 2:
        nv = min(128, n_v_rows - r0)
        blocks.append((r0, nv))
        r0 += nv - 2
    return blocks


def _col_chunks(width: int, chunk: int):
    return [(c0, min(chunk, width - c0)) for c0 in range(0, width, chunk)]


# ---------------------------------------------------------------------------
# device program
# ---------------------------------------------------------------------------

DEFAULT_CFG = dict(
    col_chunk=364,    # output columns per unit
    psum_cols=364,    # columns per PSUM bank chunk (f32, <=512)
    dd_inplace=True,  # dd overwrites ex (measured best; separate tiles -1.5%)
    io_bufs=3,
    wk_bufs=3,
    psv_bufs=2,
    ps_bufs=6,
    dma_splits=1,
    dma_splits_o=1,
    store_eng='scalar',  # output stores from Act's DMA queue (SP issues loads)
    # engine per op: 'v' = vector (DVE), 'g' = gpsimd (Pool), 'a' = Act
    # sq: 'a' -> Act Square(psum); 'v'/'g' -> tensor mult from psum
    eng=dict(ex='v', dd='g', mm='v', c1s='v', c2s='g', t='v',
             ns1='v', ns2='v', o='v', sq='a'),
)


def _cfg_key(cfg):
    return (cfg["col_chunk"], cfg["psum_cols"], cfg["io_bufs"],
            cfg["wk_bufs"], cfg["psv_bufs"], cfg["ps_bufs"],
            cfg.get("dma_splits", 1), cfg.get("dma_splits_o", 1),
            cfg.get("vy_dma", False), cfg.get("accum3", False),
            cfg.get("dd_inplace", False), cfg.get("split_frac", 0.72),
            cfg.get("store_eng", "sync"), cfg.get("fused_io", False),
            cfg.get("norm16", False),
            tuple(sorted(cfg["eng"].items())))


def _build_program(grid: int, n_cores: int, repeats: int = 1, cfg=None):
    import contextlib

    import concourse.bacc as bacc
    import concourse.tile as tile
    from concourse import mybir
    from concourse.masks import make_identity

    cfg = cfg or DEFAULT_CFG
    f32 = mybir.dt.float32
    f16 = mybir.dt.float16

    nc = bacc.Bacc()
    vband = nc.dram_tensor("vband", [3, IN_R, IN_C], f32,
                           kind="ExternalInput")
    oband = nc.dram_tensor("oband", [3, OUT_R, OUT_C], f16,
                           kind="ExternalOutput")
    dbg = {}
    for tag in cfg.get("dbg", ()):
        dbg[tag] = nc.dram_tensor(f"dbg_{tag}", [128, 1024], f32,
                                  kind="ExternalOutput")

    with tile.TileContext(nc) as tc:
        with (
            tc.tile_pool(name="io", bufs=cfg["io_bufs"]) as io,
            tc.tile_pool(name="wk", bufs=cfg["wk_bufs"]) as wk,
            tc.tile_pool(name="psv", bufs=cfg["psv_bufs"], space="PSUM") as psv,
            tc.tile_pool(name="ps", bufs=cfg["ps_bufs"], space="PSUM") as psp,
            tc.tile_pool(name="cst", bufs=1) as cst,
        ):
            eps_tile = cst.tile([128, 1], f32, tag="eps")
            nc.vector.memset(eps_tile[:, :], 1e-7)
            # f32 staging masks (memset/affine_select reject float32r),
            # then gpsimd cast-DMA into the float32r weight tiles.
            # tid[:, :128] = I; cols 128/129 zero => tid[:, 1:129] is the
            # up-shift matrix SH[k, m] = 1 iff k == m+1.
            tid32 = cst.tile([128, 130], f32, tag="tid32")
            nc.gpsimd.memset(tid32[:, :], 0.0)
            make_identity(nc, tid32[:, 0:128], nomemset=True)
            # tmix[:, 1:129][k, m] = +1 if k == m+1 else (-1 if k == m)
            tmix32 = cst.tile([128, 130], f32, tag="tmix32")
            nc.gpsimd.memset(tmix32[:, :], 0.0)
            make_identity(nc, tmix32[:, 0:128], nomemset=True)
            nc.gpsimd.affine_select(
                out=tmix32[:, 1:129], in_=tmix32[:, 1:129],
                compare_op=mybir.AluOpType.not_equal, fill=-1.0, base=0,
                pattern=[[-1, 128]], channel_multiplier=1,
            )
            tid, tmix = tid32, tmix32

            loop = tc.For_i(0, repeats, 1) if repeats > 1 else contextlib.nullcontext()
            with loop:
                _emit_body(nc, io, wk, psv, psp, eps_tile, tid, tmix,
                           vband, oband, mybir, cfg, dbg)

    nc.finalize()
    return nc


def _emit_body(nc, io, wk, psv, psp, eps_tile, tid, tmix, vband, oband,
               mybir, cfg, dbg=None):
    dbg = dbg or {}
    f32r = mybir.dt.float32r

    def dump(tag, tile, rows, cols):
        if tag in dbg:
            nc.sync.dma_start(out=dbg[tag][0:rows, 0:cols],
                              in_=tile[0:rows, 0:cols])
    f32 = mybir.dt.float32
    f16 = mybir.dt.float16
    Alu = mybir.AluOpType
    Act = mybir.ActivationFunctionType
    ENG = {"v": nc.vector, "g": nc.gpsimd, "s": None}
    eng = {k: ENG.get(v) for k, v in cfg["eng"].items()}

    frac = cfg.get("split_frac", 0.72)

    def tt(e, out, in0, in1, op):
        if e is not None:
            e.tensor_tensor(out=out, in0=in0, in1=in1, op=op)
            return
        # split the op by columns across DVE and Pool
        kk = int(out.shape[1] * frac)
        nc.vector.tensor_tensor(out=out[:, 0:kk], in0=in0[:, 0:kk],
                                in1=in1[:, 0:kk], op=op)
        nc.gpsimd.tensor_tensor(out=out[:, kk:], in0=in0[:, kk:],
                                in1=in1[:, kk:], op=op)

    nsp = cfg.get("dma_splits", 1)
    osp = cfg.get("dma_splits_o", 1)

    def split_dma(out_tile, in_ap, rows, splits):
        if splits <= 1:
            nc.sync.dma_start(out=out_tile, in_=in_ap)
            return
        step = (rows + splits - 1) // splits
        for p0 in range(0, rows, step):
            p1 = min(p0 + step, rows)
            nc.sync.dma_start(out=out_tile[p0:p1], in_=in_ap[p0:p1])

    for r0, nv in _row_blocks(IN_R):
        nq = nv - 1   # quad rows in this block
        ns = nv - 2   # output rows in this block
        for c0, w in _col_chunks(DEV_C, cfg["col_chunk"]):
            wi = w + 2   # input columns
            # ---- load the three coordinate planes -------------------------
            if cfg.get("fused_io", False):
                v3 = io.tile([nv, 3, wi], f32, tag="v3", name="v3")
                nc.sync.dma_start(
                    out=v3[:, :, :],
                    in_=vband[0:3, r0:r0 + nv,
                              c0:c0 + wi].transpose([1, 0, 2]))
                v = [v3[:, c, :] for c in range(3)]
            else:
                v = [io.tile([nv, wi], f32, tag=f"v{c}", name=f"v{c}")
                     for c in range(3)]
                for c in range(3):
                    split_dma(v[c][:, :], vband[c, r0:r0 + nv, c0:c0 + wi],
                              nv, nsp)
            # ---- vertical edges -------------------------------------------
            ey = [wk.tile([nq, wi], f32, tag=f"ey{c}", name=f"ey{c}")
                  for c in range(3)]
            if cfg.get("vy_dma", False):
                vd = [io.tile([nq, wi], f32, tag=f"vd{c}", name=f"vd{c}")
                      for c in range(3)]
                for c in range(3):
                    split_dma(vd[c][:, :],
                              vband[c, r0 + 1:r0 + nv, c0:c0 + wi], nq, nsp)
                    tt(eng.get("ey", nc.vector), ey[c][:, :], vd[c][:, :],
                       v[c][0:nq, :], Alu.subtract)
            else:
                for c in range(3):
                    for j0, pw in _col_chunks(wi, cfg["psum_cols"]):
                        pv = psv.tile([128, pw], f32, tag="psv")
                        nc.tensor.matmul(
                            out=pv[:, :], lhsT=tmix[0:nv, 1:129],
                            rhs=v[c][:, j0:j0 + pw], start=True, stop=True,
                        )
                        nc.scalar.activation(
                            out=ey[c][:, j0:j0 + pw], in_=pv[0:nq, :],
                            func=Act.Copy,
                        )
            # ---- horizontal edges -----------------------------------------
            # ---- horizontal edges -----------------------------------------
            # ---- horizontal edges -----------------------------------------
            ex = [wk.tile([nq, w + 1], f32, tag=f"ex{c}", name=f"ex{c}") for c in range(3)]
            for c in range(3):
                tt(eng["ex"], ex[c][:, :], v[c][0:nq, 1:wi],
                   v[c][0:nq, 0:w + 1], Alu.subtract)

            # ---- cross products -------------------------------------------
            # c1 = cross(ex, eyR); then ex becomes dd = ex + eyR in place;
            # c2 = cross(dd, eyL). One scratch plane per component.
            c1 = [wk.tile([nq, w + 1], f32, tag=f"c1{c}", name=f"c1{c}") for c in range(3)]
            c2 = [wk.tile([nq, w + 1], f32, tag=f"c2{c}", name=f"c2{c}") for c in range(3)]
            m2 = [wk.tile([nq, w + 1], f32, tag=f"m2{c}", name=f"m2{c}") for c in range(3)]
            for k in range(3):
                u, x = (k + 1) % 3, (k + 2) % 3
                tt(eng["mm"], c1[k][:, :], ex[u][:, :], ey[x][:, 1:wi],
                   Alu.mult)
                tt(eng["mm"], m2[k][:, :], ex[x][:, :], ey[u][:, 1:wi],
                   Alu.mult)
                tt(eng["c1s"], c1[k][:, :], c1[k][:, :], m2[k][:, :],
                   Alu.subtract)
            if cfg.get("dd_inplace", False):
                dd = ex
            else:
                dd = [wk.tile([nq, w + 1], f32, tag=f"dd{c}", name=f"dd{c}")
                      for c in range(3)]
            for c in range(3):
                tt(eng["dd"], dd[c][:, :], ex[c][:, :], ey[c][:, 1:wi],
                   Alu.add)
            for k in range(3):
                u, x = (k + 1) % 3, (k + 2) % 3
                tt(eng["mm"], c2[k][:, :], dd[u][:, :], ey[x][:, 0:w + 1],
                   Alu.mult)
                tt(eng["mm"], m2[k][:, :], dd[x][:, :], ey[u][:, 0:w + 1],
                   Alu.mult)
                tt(eng["c2s"], c2[k][:, :], c2[k][:, :], m2[k][:, :],
                   Alu.subtract)

            dump("c10", c1[0], nq, w + 1)
            dump("c20", c2[0], nq, w + 1)
            # ---- 4-term exact fp32 PE accumulation ------------------------
            # s[r][j] = t[r+1][j+1] + c1[r+1][j] + t[r][j] + c2[r][j+1]
            t = [wk.tile([nq, w + 1], f32, tag=f"t{c}", name=f"t{c}")
                 for c in range(3)]
            for k in range(3):
                tt(eng["t"], t[k][:, :], c1[k][:, :], c2[k][:, :], Alu.add)
            if cfg.get("fused_io", False):
                o3 = io.tile([ns, 3, w], f16, tag="o3", name="o3")
            accum3 = cfg.get("accum3", False)
            if accum3:
                p = [wk.tile([nq, w], f32, tag=f"p{c}", name=f"p{c}")
                     for c in range(3)]
                for k in range(3):
                    tt(eng.get("p", nc.gpsimd), p[k][:, :], t[k][:, 1:w + 1],
                       c1[k][:, 0:w], Alu.add)
            if cfg.get("fused_io", False):
                o = [o3[:, c, :] for c in range(3)]
            else:
                o = [io.tile([ns, w], f16, tag=f"o{c}", name=f"o{c}")
                     for c in range(3)]
            I_, SH = tid[0:nq, 0:128], tid[0:nq, 1:129]
            for j0, pw in _col_chunks(w, cfg["psum_cols"]):
                ps = [psp.tile([128, pw], f32, tag="ps", name="ps") for _ in range(3)]
                # grouped by stationary weight to avoid per-matmul reloads
                if accum3:
                    mm_seq = [(SH, k, p[k][:, j0:j0 + pw]) for k in range(3)]
                    nterms = 3
                else:
                    mm_seq = [(SH, k, t[k][:, j0 + 1:j0 + 1 + pw])
                              for k in range(3)]
                    mm_seq += [(SH, k, c1[k][:, j0:j0 + pw]) for k in range(3)]
                    nterms = 4
                mm_seq += [(I_, k, t[k][:, j0:j0 + pw]) for k in range(3)]
                mm_seq += [(I_, k, c2[k][:, j0 + 1:j0 + 1 + pw])
                           for k in range(3)]
                seen = [0, 0, 0]
                for lhs, k, rhs in mm_seq:
                    seen[k] += 1
                    nc.tensor.matmul(out=ps[k][:, :], lhsT=lhs, rhs=rhs,
                                     start=(seen[k] == 1),
                                     stop=(seen[k] == nterms))
                ndt = f16 if cfg.get("norm16", False) else f32
                sq = [wk.tile([ns, pw], ndt, tag=f"sq{c}", name=f"sq{c}") for c in range(3)]
                for k in range(3):
                    if cfg["eng"]["sq"] == "a":
                        nc.scalar.activation(out=sq[k][:, :],
                                             in_=ps[k][0:ns, :],
                                             func=Act.Square)
                    else:
                        tt(eng["sq"], sq[k][:, :], ps[k][0:ns, :],
                           ps[k][0:ns, :], Alu.mult)
                dump("sq0", sq[0], ns, pw)
                nsq = wk.tile([ns, pw], ndt, tag="nsq")
                tt(eng["ns1"], nsq[:, :], sq[0][:, :], sq[1][:, :], Alu.add)
                tt(eng["ns2"], nsq[:, :], nsq[:, :], sq[2][:, :], Alu.add)
                dump("nsq", nsq, ns, pw)
                rn = wk.tile([ns, pw], ndt, tag="rn")
                nc.scalar.activation(out=rn[:, :], in_=nsq[:, :],
                                     func=Act.Sqrt, bias=eps_tile[:ns, :])
                with nc.allow_low_precision(reason="radial-only error; "
                                            "rn scales a unit vector"):
                    if cfg["eng"].get("rcp", "v") == "v":
                        nc.vector.reciprocal(out=rn[:, :], in_=rn[:, :])
                    else:
                        nc.gpsimd.reciprocal(out=rn[:, :], in_=rn[:, :])
                dump("rn", rn, ns, pw)
                if cfg.get("norm16", False):
                    # s in fp16: per-component relative rounding is a
                    # <=2.4e-4 direction error regardless of |s| (unlike
                    # rounding the c-terms, which cancellation amplifies)
                    s16 = [wk.tile([ns, pw], f16, tag=f"s16{c}",
                                   name=f"s16{c}") for c in range(3)]
                    for k in range(3):
                        nc.scalar.activation(out=s16[k][:, :],
                                             in_=ps[k][0:ns, :],
                                             func=Act.Copy)
                        tt(eng["o"], o[k][:, j0:j0 + pw], s16[k][:, :],
                           rn[:, :], Alu.mult)
                else:
                    for k in range(3):
                        tt(eng["o"], o[k][:, j0:j0 + pw], ps[k][0:ns, :],
                           rn[:, :], Alu.mult)
            dump("o0", o[0], ns, w)
            ws = min(w, OUT_C - c0)
            st = {"sync": nc.sync, "scalar": nc.scalar,
                  "vector": nc.vector}[cfg.get("store_eng", "sync")]
            if cfg.get("fused_io", False):
                st.dma_start(
                    out=oband[0:3, r0:r0 + ns,
                              c0:c0 + ws].transpose([1, 0, 2]),
                    in_=o3[:, :, 0:ws])
                continue
            for k in range(3):
                if osp <= 1:
                    st.dma_start(out=oband[k, r0:r0 + ns, c0:c0 + ws],
                                 in_=o[k][:, 0:ws])
                else:
                    step = (ns + osp - 1) // osp
                    for p0 in range(0, ns, step):
                        p1 = min(p0 + step, ns)
                        st.dma_start(
                            out=oband[k, r0 + p0:r0 + p1, c0:c0 + ws],
                            in_=o[k][p0:p1, 0:ws])


_PROGRAM_CACHE: dict = {}


def _get_program(grid: int, n_cores: int, repeats: int = 1, cfg=None):
    cfg = cfg or DEFAULT_CFG
    key = (grid, n_cores, repeats, _cfg_key(cfg))
    if key not in _PROGRAM_CACHE:
        _PROGRAM_CACHE[key] = _build_program(grid, n_cores, repeats, cfg)
    return _PROGRAM_CACHE[key]


def _shard_inputs(vertices: np.ndarray):
    """fp16 SoA planes, edge-padded, sliced into the 4x2 core grid."""
    G = GRID
    V3 = vertices.reshape(G, G, 3)
    VP = np.pad(V3, ((1, 1), (1, 2), (0, 0)), mode="edge")
    VPs = np.ascontiguousarray(VP.transpose(2, 0, 1))   # [3, G+2, G+2]
    in_maps = []
    for R in range(PR):
        for C in range(PC):
            sl = VPs[:, R * BASE_R:R * BASE_R + IN_R,
                     C * BASE_C:C * BASE_C + IN_C]
            in_maps.append({"vband": np.ascontiguousarray(sl)})
    return in_maps


def _gather_output(results):
    G = GRID
    out = np.empty((G, G, 3), dtype=np.float32)
    k = 0
    for R in range(PR):
        for C in range(PC):
            ob = results[k]["oband"]    # [3, OUT_R, OUT_C] fp16
            tr = BASE_R if R < PR - 1 else OUT_R
            tc_ = BASE_C if C < PC - 1 else OUT_C
            out[R * BASE_R:R * BASE_R + tr,
                C * BASE_C:C * BASE_C + tc_, :] = (
                ob[:, :tr, :tc_].transpose(1, 2, 0).astype(np.float32))
            k += 1
    return out.reshape(G * G, 3)


def _run_stencil_on_device(vertices: np.ndarray, grid: int, n_cores: int,
                           trace: bool = False, repeats: int = 1, cfg=None):
    from concourse.bass_utils import run_bass_kernel_spmd

    in_maps = _shard_inputs(vertices)
    nc = _get_program(grid, n_cores, repeats, cfg)
    kres = run_bass_kernel_spmd(nc, in_maps, list(range(n_cores)),
                                trace=trace)
    return _gather_output(kres.results), kres


def kernel(vertices: np.ndarray, faces: np.ndarray) -> np.ndarray:
    vertices = np.asarray(vertices, dtype=np.float32)
    faces = np.asarray(faces)
    if (
        vertices.shape == (GRID * GRID, 3)
        and _is_structured(faces, GRID)
    ):
        out, _ = _run_stencil_on_device(vertices, GRID, N_CORES)
        return out
    print("kernel: faces are not the structured triangulation; host fallback",
          file=sys.stderr)
    return _host_fallback(vertices, faces)
